# revision 7
# baseline (speedup 1.0000x reference)
"""Trainium2 Bass kernel for nn_GTLayer (sparse_attention problem).

Key structural fact about the reference: H == 1 and the softmax is taken
over the HEAD axis, so softmax(attn, axis=0) on a (1, N, N) tensor is
identically 1.0.  Therefore attn @ v reduces to broadcasting the column
sums of v to every row: the A mask, q and k projections are all dead
code.  The attention output row is a single constant vector

    base = (sum_i h_i) @ vw + N * vb, then @ ow + ob

which we compute exactly on the host.  Folding both BatchNorms (eval
mode -> per-feature affine) and the residuals, the whole layer is

    y = h2 + relu(h2 @ W1 + b1) @ W2 + C        (per-feature constants)

with h2 = h * sP.  The large constant part of t = relu(h2 @ W1 + b1) is
tc = relu(b1) (h2 is zero-mean): the device computes tv = t - tc in fp8
(small values -> accurate) and the exact tc @ W2 + C contribution rides
in the bf16 h2ct tensor, added on the vector engine.  The final output
norm is dominated by the constant row (|y|_rms ~ 144), so fp8 matmul
noise lands at ~2e-3 relative - 10x under the 2e-2 gate.

Device pipeline per core (1024 rows, all matmuls fp8 e4m3 DoubleRow,
2x PE throughput).  Since tc = relu(b1), the relu + recenter chain
fuses into a single DVE op: relu(z+b1) - tc == max(z + (b1-tc), -tc).
  mm1:  zT = W1^T @ h2T            (PE, fp8 DR, psum f32) per j-chunk
  DVE:  tv = max(z + s, -tc)       (psum f32 -> sbuf fp8, one op)
  mm2:  fT = W2^T @ tv             (PE, fp8 DR; W2 stationary, output
                                    TRANSPOSED [d, row])
  DVE:  yT = fT + h2ct             (psum + sbuf bf16 -> bf16)
  DMA out (yT, bf16; host transposes back and upcasts to f32).

Rows (N=8192) are sharded over the 8 cores; weights are replicated.
Input DMAs are coalesced into 6 triggers issued critical-first on the
sync queue; a chain of warm-up matmuls keeps the PE HAM busy from t~=0
so the real matmuls run at 2.4 GHz.
"""

import numpy as np
from contextlib import ExitStack

import ml_dtypes
import concourse.bass as bass
import concourse.mybir as mybir
import concourse.tile as tile
from concourse import bacc
from concourse.bass_utils import run_bass_kernel_spmd

N = 8192
D = 512
H1 = 1024
NCORES = 8
RPC = N // NCORES  # rows per core
EPS = 1e-5

BF16 = mybir.dt.bfloat16
F32 = mybir.dt.float32
FP8 = mybir.dt.float8e4
NPBF16 = np.dtype(ml_dtypes.bfloat16)
NPFP8 = np.dtype(ml_dtypes.float8_e4m3)
DR = mybir.MatmulPerfMode.DoubleRow

KC = D // 128    # 4 k-chunks in mm1 (2 DoubleRow pairs)
NC1 = H1 // 128  # 8 j-chunks of H1 (4 DoubleRow pairs in mm2)
DS = D // 128    # 4 d-slices of the transposed mm2 output
RG = RPC // 512  # 2 row groups (matmul moving free dim 512)
N_WARMUP = 6


def build_bass():
    nc = bacc.Bacc(
        "TRN2", target_bir_lowering=False, debug=False, num_devices=NCORES
    )
    h2T = nc.dram_tensor("h2t", [D, RPC], FP8, kind="ExternalInput")
    W1 = nc.dram_tensor("w1", [D, H1], FP8, kind="ExternalInput")
    W2 = nc.dram_tensor("w2", [H1, D], FP8, kind="ExternalInput")
    H2CT = nc.dram_tensor("h2ct", [D, RPC], BF16, kind="ExternalInput")
    # b1 (cols 0..7) and tc (cols 8..15) packed: one DMA trigger
    BC = nc.dram_tensor("bc", [128, 2 * NC1], F32, kind="ExternalInput")
    YT = nc.dram_tensor("yt", [D, RPC], BF16, kind="ExternalOutput")

    with ExitStack() as ctx:
        tc = ctx.enter_context(tile.TileContext(nc))
        consts = ctx.enter_context(tc.tile_pool(name="consts", bufs=1))
        acts = ctx.enter_context(tc.tile_pool(name="acts", bufs=1))
        zpsum = ctx.enter_context(tc.tile_pool(name="zpsum", bufs=2, space="PSUM"))
        fpsum = ctx.enter_context(tc.tile_pool(name="fpsum", bufs=2, space="PSUM"))
        ypool = ctx.enter_context(tc.tile_pool(name="ypool", bufs=2))

        # --- PE warm-up on a memset tile: no DMA dependency, so the PE's
        # HAM activity window fills right after the preamble and real
        # matmuls run at 2.4 GHz instead of 1.2.  Shares a PSUM bank with
        # the (late-used) mm2 accumulator.
        wa = consts.tile([128, 512], BF16)
        nc.vector.memset(wa[:], 0.0)
        wp = fpsum.tile([128, 512], F32, tag="f0")
        for _ in range(N_WARMUP):
            nc.tensor.matmul(wp[:], wa[:, :128], wa[:], start=True, stop=True)

        # --- streaming inputs, critical-path order, few triggers ----------
        # single sync queue so the critical transfers get HBM bandwidth
        # first; each trigger costs ~650ns serial on the queue.  The tiny
        # bias tensor rides the otherwise-idle scalar queue.
        bcsb = consts.tile([128, 2 * NC1], F32)
        nc.scalar.dma_start(bcsb[:], BC[:, :])
        h2tsb = acts.tile([128, KC, RPC], FP8)
        H2Tr = h2T.rearrange("(kc p) r -> p kc r", p=128)
        nc.sync.dma_start(h2tsb[:, :, 0:512], H2Tr[:, :, 0:512])
        w1sb = consts.tile([128, KC, H1], FP8)
        W1r = W1.rearrange("(kc p) j -> p kc j", p=128)
        nc.sync.dma_start(w1sb[:, :, 0:256], W1r[:, :, 0:256])
        nc.sync.dma_start(h2tsb[:, :, 512:RPC], H2Tr[:, :, 512:RPC])
        nc.sync.dma_start(w1sb[:, :, 256:H1], W1r[:, :, 256:H1])
        w2sb = consts.tile([128, NC1, D], FP8)
        W2r = W2.rearrange("(kc p) d -> p kc d", p=128)
        nc.sync.dma_start(w2sb[:], W2r[:])
        h2ctsb = acts.tile([128, DS, RPC], BF16)
        H2CTr = H2CT.rearrange("(dc p) r -> p dc r", p=128)
        nc.sync.dma_start(h2ctsb[:, 0:1, :], H2CTr[:, 0:1, :])
        nc.sync.dma_start(h2ctsb[:, 1:DS, :], H2CTr[:, 1:DS, :])

        ssb = bcsb[:, 0:NC1]                 # b1 - tc
        ntcsb = bcsb[:, NC1 : 2 * NC1]       # -tc
        YTr = YT.rearrange("(dc p) r -> dc p r", p=128)

        # tv stored transposed: [j-in-chunk, j-chunk, row], fp8
        tvsb = acts.tile([128, NC1, RPC], FP8)

        # --- mm1: zT[j, r] = sum_k W1[k, j] h2T[k, r], fp8 DoubleRow ------
        # tc == relu(b1), so relu(z+b1)-tc == max(z + (b1-tc), -tc): the
        # relu + recentering is ONE psum->fp8 DVE op.
        for jc in range(NC1):
            for rg in range(RG):
                zp = zpsum.tile([128, 512], F32, tag=f"z{rg}")
                rs = rg * 512
                for kp in range(KC // 2):
                    nc.tensor.matmul(
                        zp[:],
                        w1sb[:, 2 * kp : 2 * kp + 2, jc * 128 : (jc + 1) * 128],
                        h2tsb[:, 2 * kp : 2 * kp + 2, rs : rs + 512],
                        start=(kp == 0),
                        stop=(kp == KC // 2 - 1),
                        perf_mode=DR,
                    )
                nc.vector.tensor_scalar(
                    tvsb[:, jc, rs : rs + 512],
                    zp[:],
                    ssb[:, jc : jc + 1],
                    ntcsb[:, jc : jc + 1],
                    mybir.AluOpType.add,
                    mybir.AluOpType.max,
                )

        # --- mm2: fT[d, r] = sum_j W2[j, d] tv[j, r], fp8 DoubleRow -------
        outq = [nc.sync, nc.scalar, nc.gpsimd]
        for dc in range(DS):
            for rg in range(RG):
                fp = fpsum.tile([128, 512], F32, tag=f"f{rg}")
                rs = rg * 512
                for kp in range(NC1 // 2):
                    nc.tensor.matmul(
                        fp[:],
                        w2sb[:, 2 * kp : 2 * kp + 2, dc * 128 : (dc + 1) * 128],
                        tvsb[:, 2 * kp : 2 * kp + 2, rs : rs + 512],
                        start=(kp == 0),
                        stop=(kp == NC1 // 2 - 1),
                        perf_mode=DR,
                    )
                ysb = ypool.tile([128, 512], BF16, tag=f"y{rg}")
                nc.vector.tensor_tensor(
                    ysb[:], fp[:], h2ctsb[:, dc, rs : rs + 512], mybir.AluOpType.add
                )
                outq[(dc * RG + rg) % 3].dma_start(YTr[dc, :, rs : rs + 512], ysb[:])
    nc.compile()
    return nc


_CACHE = {}


def _get_bass():
    if "nc" not in _CACHE:
        _CACHE["nc"] = build_bass()
    return _CACHE["nc"]


def _host_fold(inputs):
    """Fold attention shortcut + BNs into W1, b1, W2, h2, h2ct (float64)."""
    f = lambda k: inputs[k].astype(np.float64)
    h = f("h")
    a1 = f("bn1_g") / np.sqrt(f("bn1_v") + EPS)
    c1 = f("bn1_b") - f("bn1_m") * a1
    a2 = f("bn2_g") / np.sqrt(f("bn2_v") + EPS)
    c2 = f("bn2_b") - f("bn2_m") * a2

    hs = h.sum(axis=0)
    s = hs @ f("vw") + N * f("vb")          # column sums of v
    base = s @ f("ow") + f("ob")            # constant attention-out row
    d1 = base * a1 + c1                     # constant row of bn1(x)
    sP = a1 * a2

    W1 = (1.0 / a2)[:, None] * f("f1w")
    b1 = (d1 @ f("f1w") + f("f1b")).astype(np.float32)
    W2 = f("f2w") * a2[None, :]
    C = (d1 + f("f2b")) * a2 + c2

    # device computes tv = max(z + (b1-tc), -tc) in f32 (== relu(z+b1)-tc
    # since tc = relu(b1)), so use the exact same f32 constants when
    # folding tc @ W2 into h2ct
    tc = np.maximum(b1, 0.0)
    Cfull = C + tc.astype(np.float64) @ W2

    h2 = h * sP[None, :]
    pack = lambda v: v.reshape(H1 // 128, 128).T
    return {
        "W1": W1.astype(NPFP8),
        "bc": np.ascontiguousarray(
            np.concatenate([pack(b1 - tc), pack(-tc)], axis=1)
        ),
        "W2": W2.astype(NPFP8),
        "h2t": np.ascontiguousarray(h2.astype(NPFP8).T),          # [D, N]
        "h2ct": np.ascontiguousarray((h2 + Cfull[None, :]).T.astype(NPBF16)),
    }


def make_in_maps(inputs):
    hf = _host_fold(inputs)
    in_maps = []
    for c in range(NCORES):
        r0 = c * RPC
        in_maps.append(
            {
                "h2t": np.ascontiguousarray(hf["h2t"][:, r0 : r0 + RPC]),
                "h2ct": np.ascontiguousarray(hf["h2ct"][:, r0 : r0 + RPC]),
                "w1": hf["W1"],
                "w2": hf["W2"],
                "bc": hf["bc"],
            }
        )
    return in_maps


def kernel(**inputs):
    nc = _get_bass()
    in_maps = make_in_maps(inputs)
    res = run_bass_kernel_spmd(nc, in_maps, core_ids=list(range(NCORES)))
    return np.concatenate(
        [r["yt"].T.astype(np.float32) for r in res.results], axis=0
    )
